# revision 4
# baseline (speedup 1.0000x reference)
"""Trainium2 Bass kernel for batched uniform cubic B-spline evaluation.

Reference: out[b,i,o,e] = sum_c cp_pad[i,o,c] * B3(14*x[b,i,e] - c + 3),
cp padded to 18 by repeating the last control point twice; c=17 contributes
0 on x in [0,1].

16 rows per i (vs 32 in the 2-tap formulation):
  - rows c'=0..14: bump  G(u) = relu(2-|u|)^3 - 4*relu(1-|u|)^3 = 6*B3(u+2)
    at u = 14x-(c'-1), weight cp[c']/6
  - row  c'=15:    ramp  R(w) = relu(w)^3 - 3*relu(w-1)^3 = 6*(B3(w)+B3(w-1)
    +B3(w-2)) at w = 14x-12, weight cp[15]/6  (covers c=15,16,17 whose
    padded weights are all cp[15]).  R(w) = G-form with multiplier -3 at
    u = w-2 since w <= 2 on the domain.

Single-pass basis via the exact min identity (no second cube needed):
    z = 2-|u|;  relu(z)^3 - 4*relu(z-1)^3 = min(relu(z)^3, 4 - 3u^2*z)
    (proof: z^3 - (4-3u^2*z) = 4(1-|u|)^3), and for the ramp
    relu(z)^3 - 3*relu(z-1)^3 = min(relu(z)^3, 3 - (2u^2-u-1)*z).
One custom DVE op computes F = min(relu(2-|S0-C1|)^3, C0 - S1*(2-|S0-C1|))
with S0 = 14x stream, S1 = q stream (quadratic in x, from matmul),
C0 = 4/3 and C1 = gamma per partition.

Per core (batch b = core id), 16 strips of 8 i (16 rows each):
  1. per 2-strip chunk: 2 matmuls [K=32] produce u = 14x [128,512] and
     the quadratic part of q [128,512] in PSUM (moving rows per slot:
     x2h,x2m,xh,xm fp16-split); the q constant {3g^2 | 405} is folded
     into the mandatory PSUM->SBUF q move via scalar.add per-partition
     bias (single-PSUM-port rule: the DVE op may read only one PSUM
     stream)
  2. custom DVE op -> basis g fp16 [128,512] in SBUF
  3. per strip: 8 matmuls [K=32 (16 real rows, zero-padded), 128, 256]
     -> PSUM, 2 i per [128,1024] tile (cols 0:256, 512:768)
  4. PSUM->SBUF fp16 copies (DVE/ACT drain-aware balanced), out DMA per
     8 i: [128 o, 8 i, 256 e] fp16 4KB/partition lines; host transposes
     (o,i,e)->(i,o,e) and upcasts to fp32.
"""

import numpy as np

B, ID, OD, NE, NCP = 8, 128, 128, 256, 16
NCORES = 8
NSTRIP = 16   # strips of 8 i
NCHUNK = 8    # 2 strips per chunk

_cache = {}

_OP_NAME = "BSPLINE_BUMP_ANT"

# effective per-op engine costs (ns) for load balancing, incl. DVE drain
_DVE_COPY_EFF = 942.0   # [128,512] PSUM fp32 -> SBUF fp16, 2*dur-266
_ACT_COPY_EFF = 682.0


def _register_dve_op():
    """Author the fused-bump custom DVE op via the documented Spec API
    (concourse custom-DVE authoring interface; no firmware change)."""
    if "dve_op" in _cache:
        return _cache["dve_op"]
    from concourse import dve_ops
    from concourse.dve_spec import (
        Spec, Src0, Src1, C0, C1, One, relu, sq, minn, lower, AluOp, Bin,
    )
    from concourse.dve_uop import DveOpSpec

    for op in dve_ops.OPS:
        if op.name == _OP_NAME:
            _cache["dve_op"] = op
            return op

    two = One + One
    d = Bin(AluOp.ABSOLUTE_DIFF, Src0, C1)   # |14x - gamma|
    z = two - d                               # 2 - |u|
    r = relu(z)
    s = sq(r)
    cb = s * r                                # relu(z)^3
    t = Src1 * z                              # q*z
    h = C0 - t
    body = minn(cb, h)

    def _ref(in0, in1, s0, s1, imm2):
        u = in0.astype(np.float32)
        z = 2.0 - np.abs(u - s1)
        cb = np.maximum(z, 0.0) ** 3
        h = s0 - in1.astype(np.float32) * z
        return np.minimum(cb, h).astype(np.float32)

    spec = Spec(body=body, reference=_ref)
    opcode = dve_ops._CUSTOM_DVE_ROW_BASE + len(dve_ops.OPS)
    assert opcode < 0x20
    shas = {}
    for ver in ("v3", "v4"):
        uops = lower(spec, ver=ver)
        shas[ver] = DveOpSpec(name=_OP_NAME, opcode=opcode, uops=uops,
                              rd1_en=True).sha(ver)
    op = dve_ops.DveOp(_OP_NAME, spec, subdim=False, uops_sha=shas)
    dve_ops.OPS.append(op)
    dve_ops.CUSTOM_DVE_SPECS[_OP_NAME] = spec
    dve_ops._SUB_OPCODE_FOR_NAME[_OP_NAME] = opcode
    _cache["dve_op"] = op
    return op


def _build_program():
    import concourse.mybir as mybir
    import concourse.tile as tile
    from concourse import bacc

    F32 = mybir.dt.float32
    F16 = mybir.dt.float16

    bump_op = _register_dve_op()

    nc = bacc.Bacc("TRN2", target_bir_lowering=False)
    w_d = nc.dram_tensor("w", [128, 16 * 256], F16, kind="ExternalInput")
    x3_d = nc.dram_tensor("x3", [32, 16 * 256], F16, kind="ExternalInput")
    selu_d = nc.dram_tensor("selu", [32, 128], F16, kind="ExternalInput")
    selq_d = nc.dram_tensor("selq", [32, 128], F16, kind="ExternalInput")
    cc_d = nc.dram_tensor("cc", [128, 3], F32, kind="ExternalInput")
    out_d = nc.dram_tensor("out", [128, 128, 256], F16, kind="ExternalOutput")

    with tile.TileContext(nc) as tc:
        with (
            tc.tile_pool(name="const", bufs=1) as cpool,
            tc.tile_pool(name="gp", bufs=2) as gpool,
            tc.tile_pool(name="obp", bufs=3) as obpool,
            tc.tile_pool(name="uqp", bufs=1, space="PSUM") as uqpool,
            tc.tile_pool(name="mmp", bufs=1, space="PSUM") as mmpool,
        ):
            # loads: criticals first (sels + consts + first x3/w chunks)
            selu_t = cpool.tile([32, 128], F16)
            nc.sync.dma_start(out=selu_t[:], in_=selu_d.ap())
            selq_t = cpool.tile([32, 128], F16)
            nc.sync.dma_start(out=selq_t[:], in_=selq_d.ap())
            cc_t = cpool.tile([128, 3], F32)
            nc.sync.dma_start(out=cc_t[:], in_=cc_d.ap())
            x3_t = cpool.tile([32, 16 * 256], F16)
            nc.sync.dma_start(out=x3_t[:, 0:512], in_=x3_d.ap()[:, 0:512])
            w_t = cpool.tile([128, 16 * 256], F16)
            nc.sync.dma_start(out=w_t[:, 0:1024], in_=w_d.ap()[:, 0:1024])
            for xc in range(1, 8):
                nc.sync.dma_start(out=x3_t[:, xc * 512:(xc + 1) * 512],
                                  in_=x3_d.ap()[:, xc * 512:(xc + 1) * 512])
            for wc in range(1, 4):
                nc.sync.dma_start(out=w_t[:, wc * 1024:(wc + 1) * 1024],
                                  in_=w_d.ap()[:, wc * 1024:(wc + 1) * 1024])

            eng_ns = {"dve": 0.0, "act": 0.0}

            def copy_balanced(dst, src):
                if eng_ns["dve"] + _DVE_COPY_EFF <= eng_ns["act"] + _ACT_COPY_EFF:
                    nc.vector.tensor_copy(dst, src)
                    eng_ns["dve"] += _DVE_COPY_EFF
                else:
                    nc.scalar.copy(dst, src)
                    eng_ns["act"] += _ACT_COPY_EFF

            def mk_chunk(c):
                u_t = uqpool.tile([128, 512], F32, tag="u", name=f"u_{c}")
                q_t = uqpool.tile([128, 512], F32, tag="q", name=f"q_{c}")
                qs_t = gpool.tile([128, 512], F32, tag="qs", name=f"qs_{c}")
                g_t = gpool.tile([128, 512], F16, tag="g", name=f"g_{c}")
                c0, c1 = c * 512, (c + 1) * 512

                def op_qcopy():
                    # single-PSUM-port rule: basis op can read only one PSUM
                    # stream, so q moves to SBUF on the Scalar engine; the
                    # per-partition q constant rides along as activation bias
                    nc.scalar.add(qs_t[:], q_t[:], cc_t[:, 2:3])
                    eng_ns["act"] += _ACT_COPY_EFF

                def op_basis():
                    nc.vector._custom_dve(
                        bump_op, out=g_t[:], in0=u_t[:], in1=qs_t[:],
                        s0=cc_t[:, 0:1], s1=cc_t[:, 1:2])
                    eng_ns["dve"] += 2 * 658.0 - 266.0

                ops = [
                    lambda: nc.tensor.matmul(
                        q_t[:], selq_t[:], x3_t[:, c0:c1],
                        start=True, stop=True, tile_position=(0, 0)),
                    lambda: nc.tensor.matmul(
                        u_t[:], selu_t[:], x3_t[:, c0:c1],
                        start=True, stop=True, tile_position=(0, 0)),
                    op_qcopy,
                    op_basis,
                ]
                return g_t, ops

            def emit_strip(s, g_t, pend):
                h = s % 2
                ob = obpool.tile([128, 8 * 256], F16, tag="ob", name=f"ob_{s}")
                for m in range(4):
                    ps = mmpool.tile([128, 1024], F32,
                                     tag=f"o{(4 * s + m) % 3}",
                                     name=f"ps_{s}_{m}")
                    for j in (0, 1):
                        icol = s * 256 + j * 128
                        nc.tensor.matmul(
                            ps[:, j * 512:j * 512 + 256],
                            w_t[32 * m:32 * m + 32, icol:icol + 128],
                            g_t[32 * m:32 * m + 32, h * 256:(h + 1) * 256],
                            start=True, stop=True,
                            tile_position=(32 * m, 0),
                        )
                    src = ps[:].rearrange("p (b e) -> p b e", e=512)[:, :, 0:256]
                    dst = ob[:, 2 * m * 256:(2 * m + 2) * 256].rearrange(
                        "p (b e) -> p b e", e=256)
                    copy_balanced(dst, src)
                    if pend:
                        pend.pop(0)()
                nc.sync.dma_start(
                    out=out_d.ap()[:, 8 * s:8 * s + 8, :],
                    in_=ob[:].rearrange("o (i e) -> o i e", e=256))
                for op in pend:
                    op()

            g_t, ops0 = mk_chunk(0)
            for op in ops0:
                op()
            for c in range(NCHUNK):
                pend = []
                if c + 1 < NCHUNK:
                    g_next, pend = mk_chunk(c + 1)
                emit_strip(2 * c, g_t, [])
                emit_strip(2 * c + 1, g_t, list(pend))
                if c + 1 < NCHUNK:
                    g_t = g_next
    nc.finalize()
    return nc


def _host_prep(cp):
    """Build w (fp16 weights, zero-padded 32-row tiles), sel_u, sel_q, cc."""
    cpd = np.asarray(cp, np.float64)
    w6 = cpd / 6.0                                   # (128 i, 128 o, 16 c')
    w_host = np.zeros((128, 4096), np.float16)
    for i in range(ID):
        s, r = divmod(i, 8)
        m, j = divmod(r, 2)
        col = 256 * s + 128 * j
        rows = 32 * m + 16 * j
        # (16 c', 128 o) block
        w_host[rows:rows + 16, col:col + 128] = w6[i].T.astype(np.float16)

    selu = np.zeros((32, 128), np.float16)
    selq = np.zeros((32, 128), np.float16)
    cc = np.zeros((128, 3), np.float32)
    for q in range(8):
        for cpr in range(16):
            p = 16 * q + cpr
            if cpr < 15:
                gam = cpr - 1.0
                c0v, x2c, xc, onec = 4.0, 588.0, -84.0 * gam, 3.0 * gam * gam
            else:
                gam = 14.0
                c0v, x2c, xc, onec = 3.0, 392.0, -798.0, 405.0
            cc[p, 0] = c0v
            cc[p, 1] = gam
            cc[p, 2] = onec
            selu[4 * q + 2, p] = 14.0
            selu[4 * q + 3, p] = 14.0
            selq[4 * q + 0, p] = x2c
            selq[4 * q + 1, p] = x2c
            selq[4 * q + 2, p] = xc
            selq[4 * q + 3, p] = xc
    return w_host, selu, selq, cc


def _make_x3(xb):
    """x3 [32, 4096] fp16: rows 4q+{0..3} = x2h,x2m,xh,xm for i=8s+q at
    cols 256s:256s+256."""
    xb = np.asarray(xb, np.float64)                  # (128 i, 256 e)
    x2 = xb * xb
    x2h = x2.astype(np.float16)
    x2m = (x2 - x2h.astype(np.float64)).astype(np.float16)
    xh = xb.astype(np.float16)
    xm = (xb - xh.astype(np.float64)).astype(np.float16)
    x3 = np.zeros((32, 4096), np.float16)
    v = x3.reshape(8, 4, 16, 256)                    # (q, row, s, e)
    src = (x2h, x2m, xh, xm)
    for rr in range(4):
        # arr (128 i, 256 e) -> (16 s, 8 q, 256 e) -> (8 q, 16 s, 256 e)
        v[:, rr] = src[rr].reshape(16, 8, 256).transpose(1, 0, 2)
    return x3


def kernel(x, cp, k, _trace=False, _tmpdir=None):
    from concourse.bass_utils import run_bass_kernel_spmd

    x = np.asarray(x, dtype=np.float32)
    cp = np.asarray(cp, dtype=np.float32)
    assert int(k) == 3, "kernel hardcoded for cubic (k=3)"
    assert x.shape == (B, ID, NE) and cp.shape == (ID, OD, NCP)

    w_host, selu, selq, cc = _host_prep(cp)
    in_maps = [{"w": w_host, "x3": _make_x3(x[c]), "selu": selu,
                "selq": selq, "cc": cc} for c in range(NCORES)]

    if "nc" not in _cache:
        _cache["nc"] = _build_program()
    nc = _cache["nc"]

    kwargs = {}
    if _trace:
        kwargs = {"trace": True, "tmpdir": _tmpdir,
                  "trace_cores": list(range(NCORES))}
    res = run_bass_kernel_spmd(nc, in_maps, core_ids=list(range(NCORES)), **kwargs)
    out = np.stack([res.results[c]["out"].swapaxes(0, 1) for c in range(NCORES)],
                   axis=0).astype(np.float32)
    if _trace:
        kernel.last_result = res
    return out


# revision 6
# speedup vs baseline: 1.0063x; 1.0063x over previous
"""Trainium2 Bass kernel for batched uniform cubic B-spline evaluation.

Reference: out[b,i,o,e] = sum_c cp_pad[i,o,c] * B3(14*x[b,i,e] - c + 3),
cp padded to 18 by repeating the last control point twice; c=17 contributes
0 on x in [0,1].

16 rows per i (vs 32 in the 2-tap formulation):
  - rows c'=0..14: bump  G(u) = relu(2-|u|)^3 - 4*relu(1-|u|)^3 = 6*B3(u+2)
    at u = 14x-(c'-1), weight cp[c']/6
  - row  c'=15:    ramp  R(w) = relu(w)^3 - 3*relu(w-1)^3 = 6*(B3(w)+B3(w-1)
    +B3(w-2)) at w = 14x-12, weight cp[15]/6  (covers c=15,16,17 whose
    padded weights are all cp[15]).  R(w) = G-form with multiplier -3 at
    u = w-2 since w <= 2 on the domain.

Single-pass basis via the exact min identity (no second cube needed):
    z = 2-|u|;  relu(z)^3 - 4*relu(z-1)^3 = min(relu(z)^3, 4 - 3u^2*z)
    (proof: z^3 - (4-3u^2*z) = 4(1-|u|)^3), and for the ramp
    relu(z)^3 - 3*relu(z-1)^3 = min(relu(z)^3, 3 - (2u^2-u-1)*z).
One custom DVE op computes F = min(relu(2-|S0-C1|)^3, C0 - S1*(2-|S0-C1|))
with S0 = 14x stream, S1 = q stream (quadratic in x, from matmul),
C0 = 4/3 and C1 = gamma per partition.

Per core (batch b = core id), 16 strips of 8 i (16 rows each):
  1. per 2-strip chunk: 2 matmuls [K=32,128,512] produce u = 14x and the
     quadratic part of q in PSUM (moving rows per slot: x2h,x2m,xh,xm
     fp16-split); the q constant {3g^2 | 405} folds into the mandatory
     PSUM->SBUF q move via scalar.add per-partition bias (single-PSUM-
     port rule: the DVE op may read only one PSUM stream)
  2. custom DVE op -> basis g fp16 [128,512] in SBUF
  3. per strip: 8 matmuls [K=32 (16 real rows, zero-padded), 128, 256]
     -> 2 PSUM tiles [128,1024], 4 i packed contiguously
  4. contiguous [128,1024] PSUM->SBUF fp16 copies (DVE/ACT balanced);
     out DMA per 2 strips: [128 o, 16 i, 256 e] fp16 8KB/partition
     lines; host transposes (o,i,e)->(i,o,e) and upcasts to fp32.
Input loads are batched into 5 dma_starts split across the sync and
(idle) gpsimd queues to cut serial descriptor-gen startup time.
"""

import numpy as np

B, ID, OD, NE, NCP = 8, 128, 128, 256, 16
NCORES = 8
NSTRIP = 16   # strips of 8 i
NCHUNK = 8    # 2 strips per chunk

_cache = {}

_OP_NAME = "BSPLINE_BUMP_ANT"

# measured per-op engine costs (ns) for load balancing
_DVE_COPY1024 = 1240.0   # [128,1024] PSUM fp32 -> SBUF fp16 tensor_copy
_ACT_COPY1024 = 1050.0   # [128,1024] PSUM fp32 -> SBUF fp16 scalar copy
_DVE_BASIS = 800.0
_ACT_QCOPY = 650.0


def _register_dve_op():
    """Author the fused-bump custom DVE op via the documented Spec API
    (concourse custom-DVE authoring interface; no firmware change)."""
    if "dve_op" in _cache:
        return _cache["dve_op"]
    from concourse import dve_ops
    from concourse.dve_spec import (
        Spec, Src0, Src1, C0, C1, One, relu, sq, minn, lower, AluOp, Bin,
    )
    from concourse.dve_uop import DveOpSpec

    for op in dve_ops.OPS:
        if op.name == _OP_NAME:
            _cache["dve_op"] = op
            return op

    two = One + One
    d = Bin(AluOp.ABSOLUTE_DIFF, Src0, C1)   # |14x - gamma|
    z = two - d                               # 2 - |u|
    r = relu(z)
    s = sq(r)
    cb = s * r                                # relu(z)^3
    t = Src1 * z                              # q*z
    h = C0 - t
    body = minn(cb, h)

    def _ref(in0, in1, s0, s1, imm2):
        u = in0.astype(np.float32)
        z = 2.0 - np.abs(u - s1)
        cb = np.maximum(z, 0.0) ** 3
        h = s0 - in1.astype(np.float32) * z
        return np.minimum(cb, h).astype(np.float32)

    spec = Spec(body=body, reference=_ref)
    opcode = dve_ops._CUSTOM_DVE_ROW_BASE + len(dve_ops.OPS)
    assert opcode < 0x20
    shas = {}
    for ver in ("v3", "v4"):
        uops = lower(spec, ver=ver)
        shas[ver] = DveOpSpec(name=_OP_NAME, opcode=opcode, uops=uops,
                              rd1_en=True).sha(ver)
    op = dve_ops.DveOp(_OP_NAME, spec, subdim=False, uops_sha=shas)
    dve_ops.OPS.append(op)
    dve_ops.CUSTOM_DVE_SPECS[_OP_NAME] = spec
    dve_ops._SUB_OPCODE_FOR_NAME[_OP_NAME] = opcode
    _cache["dve_op"] = op
    return op


def _build_program():
    import concourse.mybir as mybir
    import concourse.tile as tile
    from concourse import bacc

    F32 = mybir.dt.float32
    F16 = mybir.dt.float16

    bump_op = _register_dve_op()

    nc = bacc.Bacc("TRN2", target_bir_lowering=False)
    w_d = nc.dram_tensor("w", [128, 16 * 256], F16, kind="ExternalInput")
    x3_d = nc.dram_tensor("x3", [32, 16 * 256], F16, kind="ExternalInput")
    sel2_d = nc.dram_tensor("sel2", [32, 256], F16, kind="ExternalInput")
    cc_d = nc.dram_tensor("cc", [128, 3], F32, kind="ExternalInput")
    out_d = nc.dram_tensor("out", [128, 128, 256], F16, kind="ExternalOutput")

    with tile.TileContext(nc) as tc:
        with (
            tc.tile_pool(name="const", bufs=1) as cpool,
            tc.tile_pool(name="gp", bufs=2) as gpool,
            tc.tile_pool(name="obp", bufs=2) as obpool,
            tc.tile_pool(name="uqp", bufs=1, space="PSUM") as uqpool,
            tc.tile_pool(name="mmp", bufs=1, space="PSUM") as mmpool,
        ):
            # batched loads; gpsimd queue (idle engine) takes the bulk so
            # the sync queue's serial descriptor-gen doesn't gate startup
            sel2_t = cpool.tile([32, 256], F16)
            nc.sync.dma_start(out=sel2_t[:], in_=sel2_d.ap())
            cc_t = cpool.tile([128, 3], F32)
            nc.sync.dma_start(out=cc_t[:], in_=cc_d.ap())
            x3_t = cpool.tile([32, 16 * 256], F16)
            nc.gpsimd.dma_start(out=x3_t[:, 0:2048], in_=x3_d.ap()[:, 0:2048])
            w_t = cpool.tile([128, 16 * 256], F16)
            nc.sync.dma_start(out=w_t[:, 0:2048], in_=w_d.ap()[:, 0:2048])
            nc.gpsimd.dma_start(out=x3_t[:, 2048:4096],
                                in_=x3_d.ap()[:, 2048:4096])
            nc.gpsimd.dma_start(out=w_t[:, 2048:4096],
                                in_=w_d.ap()[:, 2048:4096])
            selu_t = sel2_t[:, 0:128]
            selq_t = sel2_t[:, 128:256]

            eng_ns = {"dve": 0.0, "act": 0.0}

            def copy_balanced(dst, src):
                if eng_ns["dve"] + _DVE_COPY1024 <= eng_ns["act"] + _ACT_COPY1024:
                    nc.vector.tensor_copy(dst, src)
                    eng_ns["dve"] += _DVE_COPY1024
                else:
                    nc.scalar.copy(dst, src)
                    eng_ns["act"] += _ACT_COPY1024

            def mk_chunk(c):
                u_t = uqpool.tile([128, 512], F32, tag="u", name=f"u_{c}")
                q_t = uqpool.tile([128, 512], F32, tag="q", name=f"q_{c}")
                qs_t = gpool.tile([128, 512], F32, tag="qs", name=f"qs_{c}")
                g_t = gpool.tile([128, 512], F16, tag="g", name=f"g_{c}")
                c0, c1 = c * 512, (c + 1) * 512

                def op_qcopy():
                    # single-PSUM-port rule: the basis op can read only one
                    # PSUM stream, so q moves to SBUF on the Scalar engine;
                    # the per-partition q constant rides along as the bias
                    nc.scalar.add(qs_t[:], q_t[:], cc_t[:, 2:3])
                    eng_ns["act"] += _ACT_QCOPY

                def op_basis():
                    nc.vector._custom_dve(
                        bump_op, out=g_t[:], in0=u_t[:], in1=qs_t[:],
                        s0=cc_t[:, 0:1], s1=cc_t[:, 1:2])
                    eng_ns["dve"] += _DVE_BASIS

                ops = [
                    lambda: nc.tensor.matmul(
                        q_t[:], selq_t, x3_t[:, c0:c1],
                        start=True, stop=True, tile_position=(0, 0)),
                    lambda: nc.tensor.matmul(
                        u_t[:], selu_t, x3_t[:, c0:c1],
                        start=True, stop=True, tile_position=(0, 0)),
                    op_qcopy,
                    op_basis,
                ]
                return g_t, ops

            def emit_strip(s, g_t, ob, pend):
                h = s % 2
                oc0 = h * 2048
                for t in range(2):
                    ps = mmpool.tile([128, 1024], F32,
                                     tag=f"o{(2 * s + t) % 3}",
                                     name=f"ps_{s}_{t}")
                    for v in range(4):
                        m, j = divmod(4 * t + v, 2)
                        icol = s * 256 + j * 128
                        nc.tensor.matmul(
                            ps[:, v * 256:(v + 1) * 256],
                            w_t[32 * m:32 * m + 32, icol:icol + 128],
                            g_t[32 * m:32 * m + 32, h * 256:(h + 1) * 256],
                            start=True, stop=True,
                            tile_position=(32 * m, 0),
                        )
                    copy_balanced(ob[:, oc0 + t * 1024:oc0 + (t + 1) * 1024],
                                  ps[:])
                    if pend:
                        pend.pop(0)()

            g_t, ops0 = mk_chunk(0)
            for op in ops0:
                op()
            for c in range(NCHUNK):
                pend = []
                if c + 1 < NCHUNK:
                    g_next, pend = mk_chunk(c + 1)
                ob = obpool.tile([128, 16 * 256], F16, tag="ob", name=f"ob_{c}")
                emit_strip(2 * c, g_t, ob, [])
                emit_strip(2 * c + 1, g_t, ob, list(pend))
                nc.sync.dma_start(
                    out=out_d.ap()[:, 16 * c:16 * c + 16, :],
                    in_=ob[:].rearrange("o (i e) -> o i e", e=256))
                for op in pend:
                    op()
                if c + 1 < NCHUNK:
                    g_t = g_next
    nc.finalize()
    return nc


def _host_prep(cp):
    """Build w (fp16 weights, zero-padded 32-row tiles), sel2, cc."""
    cpd = np.asarray(cp, np.float64)
    w6 = cpd / 6.0                                   # (128 i, 128 o, 16 c')
    w_host = np.zeros((128, 4096), np.float16)
    for i in range(ID):
        s, r = divmod(i, 8)
        m, j = divmod(r, 2)
        col = 256 * s + 128 * j
        rows = 32 * m + 16 * j
        w_host[rows:rows + 16, col:col + 128] = w6[i].T.astype(np.float16)

    sel2 = np.zeros((32, 256), np.float16)
    cc = np.zeros((128, 3), np.float32)
    for q in range(8):
        for cpr in range(16):
            p = 16 * q + cpr
            if cpr < 15:
                gam = cpr - 1.0
                c0v, x2c, xc, onec = 4.0, 588.0, -84.0 * gam, 3.0 * gam * gam
            else:
                gam = 14.0
                c0v, x2c, xc, onec = 3.0, 392.0, -798.0, 405.0
            cc[p, 0] = c0v
            cc[p, 1] = gam
            cc[p, 2] = onec
            sel2[4 * q + 2, p] = 14.0          # selu
            sel2[4 * q + 3, p] = 14.0
            sel2[4 * q + 0, 128 + p] = x2c     # selq
            sel2[4 * q + 1, 128 + p] = x2c
            sel2[4 * q + 2, 128 + p] = xc
            sel2[4 * q + 3, 128 + p] = xc
    return w_host, sel2, cc


def _make_x3(xb):
    """x3 [32, 4096] fp16: rows 4q+{0..3} = x2h,x2m,xh,xm for i=8s+q at
    cols 256s:256s+256."""
    xb = np.asarray(xb, np.float64)                  # (128 i, 256 e)
    x2 = xb * xb
    x2h = x2.astype(np.float16)
    x2m = (x2 - x2h.astype(np.float64)).astype(np.float16)
    xh = xb.astype(np.float16)
    xm = (xb - xh.astype(np.float64)).astype(np.float16)
    x3 = np.zeros((32, 4096), np.float16)
    v = x3.reshape(8, 4, 16, 256)                    # (q, row, s, e)
    src = (x2h, x2m, xh, xm)
    for rr in range(4):
        # arr (128 i, 256 e) -> (16 s, 8 q, 256 e) -> (8 q, 16 s, 256 e)
        v[:, rr] = src[rr].reshape(16, 8, 256).transpose(1, 0, 2)
    return x3


def kernel(x, cp, k, _trace=False, _tmpdir=None):
    from concourse.bass_utils import run_bass_kernel_spmd

    x = np.asarray(x, dtype=np.float32)
    cp = np.asarray(cp, dtype=np.float32)
    assert int(k) == 3, "kernel hardcoded for cubic (k=3)"
    assert x.shape == (B, ID, NE) and cp.shape == (ID, OD, NCP)

    w_host, sel2, cc = _host_prep(cp)
    in_maps = [{"w": w_host, "x3": _make_x3(x[c]), "sel2": sel2, "cc": cc}
               for c in range(NCORES)]

    if "nc" not in _cache:
        _cache["nc"] = _build_program()
    nc = _cache["nc"]

    kwargs = {}
    if _trace:
        kwargs = {"trace": True, "tmpdir": _tmpdir,
                  "trace_cores": list(range(NCORES))}
    res = run_bass_kernel_spmd(nc, in_maps, core_ids=list(range(NCORES)), **kwargs)
    out = np.stack([res.results[c]["out"].swapaxes(0, 1) for c in range(NCORES)],
                   axis=0).astype(np.float32)
    if _trace:
        kernel.last_result = res
    return out
